# revision 5
# baseline (speedup 1.0000x reference)
"""CSPN 3x3 propagation step on 8 Trainium2 NeuronCores.

out[b,0,r,c] = sum_k aff[b,k,r,c] * patch_k(cur)[r,c], with the center tap
(k=4) taken from coarse_seg instead of cur_seg. Zero padding at image edges.

Sharding: pure data parallel over batch (16 images -> 2 per core), one SPMD
Bass program run on all 8 cores with per-core input slices.

Per-core algorithm (per 512x512 image, packed as [128 partitions, 4 row
blocks, 512 cols]):
  - The tap row-shift (dy) is folded into the affinity DMA: plane k is
    loaded with a source row offset of -dy_k (A'_k[s] = aff_k[s-dy]).
    The overhanging first/last source row of the shifted window lands in
    an adjacent affinity plane (never out of bounds) and its product is
    provably never consumed.
  - The tap col-shift (dx) is a free-dim offset into a column-padded cur
    tile.
  - VectorEngine computes the 9 elementwise products P_k = A'_k * cur_x,
    then per-dy-group sums V_g (2 adds per group; optionally on GpSimd).
  - TensorEngine realigns the dy groups with shift-matrix matmuls
    (multiply by exact 0/1 -> bit-exact) accumulating in PSUM, including
    the cross-block boundary rows.
  - ScalarEngine evacuates PSUM -> SBUF; DMA stores the result.
"""

import os
import sys

import numpy as np

if "/opt/trn_rl_repo" not in sys.path:
    sys.path.insert(0, "/opt/trn_rl_repo")

B_PER_CORE = 2
N_CORES = 8
H = 512
W = 512
NBLK = H // 128
WPAD = W + 2  # zero column on each side

# which engine runs the per-dy-group sums: "vector" or "gpsimd"
ADD_ENGINE = os.environ.get("CSPN_ADD_ENGINE", "gpsimd")

_compiled = None


def _shift_mats():
    """[128, 5, 128] f32: j=0 I, 1 Sd (k=m-1), 2 Su (k=m+1), 3 Ed, 4 Eu."""
    m = np.zeros((128, 5, 128), dtype=np.float32)
    for i in range(128):
        m[i, 0, i] = 1.0  # identity
    for i in range(127):
        m[i, 1, i + 1] = 1.0  # Sd: out[m] = in[m-1]
        m[i + 1, 2, i] = 1.0  # Su: out[m] = in[m+1]
    m[127, 3, 0] = 1.0  # Ed: out[0] = in[127]   (prev block)
    m[0, 4, 127] = 1.0  # Eu: out[127] = in[0]   (next block)
    return m


def _build_program():
    import concourse.bacc as bacc
    import concourse.mybir as mybir
    import concourse.tile as tile

    fp32 = mybir.dt.float32

    nc = bacc.Bacc(
        "TRN2",
        target_bir_lowering=False,
        debug=False,
        enable_asserts=False,
        num_devices=N_CORES,
    )

    aff_d = nc.dram_tensor(
        "affinity", [B_PER_CORE, 9, H, W], fp32, kind="ExternalInput"
    ).ap()
    cur_d = nc.dram_tensor(
        "cur_seg", [B_PER_CORE, 1, H, W], fp32, kind="ExternalInput"
    ).ap()
    coa_d = nc.dram_tensor(
        "coarse_seg", [B_PER_CORE, 1, H, W], fp32, kind="ExternalInput"
    ).ap()
    smat_d = nc.dram_tensor("smats", [128, 5, 128], fp32, kind="ExternalInput").ap()
    out_d = nc.dram_tensor(
        "out", [B_PER_CORE, 1, H, W], fp32, kind="ExternalOutput"
    ).ap()

    with tile.TileContext(nc) as tc:
        with (
            tc.tile_pool(name="smat", bufs=1) as smat_pool,
            tc.tile_pool(name="aff", bufs=6) as aff_pool,
            tc.tile_pool(name="prod", bufs=8) as prod_pool,
            tc.tile_pool(name="cur", bufs=2) as cur_pool,
            tc.tile_pool(name="coa", bufs=2) as coa_pool,
            tc.tile_pool(name="acc", bufs=2) as acc_pool,
            tc.tile_pool(name="psum", bufs=8, space="PSUM") as psum_pool,
        ):
            tS = smat_pool.tile([128, 5, 128], fp32)
            nc.scalar.dma_start(out=tS[:], in_=smat_d[:])
            SM_I, SM_SD, SM_SU, SM_ED, SM_EU = (tS[:, j, :] for j in range(5))

            for b in range(B_PER_CORE):
                # --- cur tile [128, 4, 514], data in cols 1..512 ---
                # cur/coarse/out ride the ACT HWDGE ring; affinity rides the
                # SP ring, so the two streams overlap.
                tM = cur_pool.tile([128, NBLK, WPAD], fp32, tag="cur")
                nc.vector.memset(tM[:, :, 0:1], 0.0)
                nc.vector.memset(tM[:, :, WPAD - 1 : WPAD], 0.0)
                nc.scalar.dma_start(
                    out=tM[:, :, 1 : W + 1],
                    in_=cur_d[b, 0].rearrange("(t p) c -> p t c", p=128),
                )

                tC = coa_pool.tile([128, NBLK, W], fp32, tag="coa")
                nc.scalar.dma_start(
                    out=tC[:], in_=coa_d[b, 0].rearrange("(t p) c -> p t c", p=128)
                )

                aff_flat = aff_d[b].flatten_outer_dims()  # [9*512, 512]

                # --- products and per-dy-group sums ---
                V = []  # V[g] for dy = g-1
                for g in range(3):
                    dy = g - 1
                    Pg = []
                    for dxi in range(3):
                        k = 3 * g + dxi
                        dx = dxi - 1
                        # per-plane DMA, source rows shifted by -dy
                        ak = aff_pool.tile([128, NBLK, W], fp32, tag="aff")
                        start = 512 * k - dy
                        nc.sync.dma_start(
                            out=ak[:],
                            in_=aff_flat[start : start + H, :].rearrange(
                                "(t p) c -> p t c", p=128
                            ),
                        )
                        pk = prod_pool.tile([128, NBLK, W], fp32, tag="prod")
                        src = tC[:] if k == 4 else tM[:, :, 1 + dx : 1 + dx + W]
                        nc.vector.tensor_mul(out=pk[:], in0=ak[:], in1=src)
                        Pg.append(pk)
                    # last group's adds on DVE (idle at the tail) to shorten
                    # the critical path; earlier groups on GpSimd in parallel
                    add_eng = nc.vector if g == 2 else nc.gpsimd
                    add_eng.tensor_add(out=Pg[0][:], in0=Pg[0][:], in1=Pg[1][:])
                    add_eng.tensor_add(out=Pg[0][:], in0=Pg[0][:], in1=Pg[2][:])
                    V.append(Pg[0])

                Vm1, V0, Vp1 = V[0], V[1], V[2]

                # --- PE: realign dy groups into psum, accumulate ---
                acc = acc_pool.tile([128, NBLK, W], fp32, tag="acc")
                out_blocks = out_d[b, 0].rearrange("(t p) c -> p t c", p=128)
                for t in range(NBLK):
                    mms = [
                        (SM_I, V0[:, t, :]),
                        (SM_SD, Vm1[:, t, :]),
                        (SM_SU, Vp1[:, t, :]),
                    ]
                    if t > 0:
                        mms.append((SM_ED, Vm1[:, t - 1, :]))
                    if t < NBLK - 1:
                        mms.append((SM_EU, Vp1[:, t + 1, :]))
                    pt = psum_pool.tile([128, W], fp32, tag="psum")
                    for i, (lhsT, rhs) in enumerate(mms):
                        nc.tensor.matmul(
                            pt[:],
                            lhsT,
                            rhs,
                            start=(i == 0),
                            stop=(i == len(mms) - 1),
                        )
                    nc.scalar.copy(out=acc[:, t, :], in_=pt[:])
                    nc.scalar.dma_start(out=out_blocks[:, t, :], in_=acc[:, t, :])

    nc.compile()
    return nc


def _get_program():
    global _compiled
    if _compiled is None:
        _compiled = _build_program()
    return _compiled


def _in_maps(affinity, cur_seg, coarse_seg):
    smats = _shift_mats()
    maps = []
    for j in range(N_CORES):
        s = slice(j * B_PER_CORE, (j + 1) * B_PER_CORE)
        maps.append(
            {
                "affinity": np.ascontiguousarray(affinity[s]),
                "cur_seg": np.ascontiguousarray(cur_seg[s]),
                "coarse_seg": np.ascontiguousarray(coarse_seg[s]),
                "smats": smats,
            }
        )
    return maps


def kernel(affinity, cur_seg, coarse_seg, i=None, **_unused):
    from concourse.bass_utils import run_bass_kernel_spmd

    nc = _get_program()

    affinity = np.ascontiguousarray(affinity, dtype=np.float32)
    cur_seg = np.ascontiguousarray(cur_seg, dtype=np.float32)
    coarse_seg = np.ascontiguousarray(coarse_seg, dtype=np.float32)

    res = run_bass_kernel_spmd(
        nc, _in_maps(affinity, cur_seg, coarse_seg), core_ids=list(range(N_CORES))
    )
    out = np.concatenate([r["out"] for r in res.results], axis=0)
    return out


# revision 12
# speedup vs baseline: 36.0878x; 36.0878x over previous
"""CSPN 3x3 propagation step on 8 Trainium2 NeuronCores.

out[b,0,r,c] = sum_k aff[b,k,r,c] * patch_k(cur)[r,c], with the center tap
(k=4) taken from coarse_seg instead of cur_seg. Zero padding at image edges.

Sharding: pure data parallel over batch (16 images -> 2 per core), one SPMD
Bass program run on all 8 cores with per-core input slices.

Per-core algorithm (per 512x512 image, packed as [128 partitions, 4 row
blocks, 512 cols]):
  - The tap row-shift (dy) is folded into the affinity DMA: plane k is
    loaded with a source row offset of -dy_k (A'_k[s] = aff_k[s-dy]).
    The overhanging first/last source row of the shifted window lands in
    an adjacent affinity plane (never out of bounds) and its product is
    provably never consumed.
  - The tap col-shift (dx) is a free-dim offset into a column-padded cur
    tile.
  - VectorEngine computes the 9 elementwise products P_k = A'_k * cur_x,
    then per-dy-group sums V_g (2 adds per group; optionally on GpSimd).
  - TensorEngine realigns the dy groups with shift-matrix matmuls
    (multiply by exact 0/1 -> bit-exact) accumulating in PSUM, including
    the cross-block boundary rows.
  - ScalarEngine evacuates PSUM -> SBUF; DMA stores the result.
"""

import sys

import numpy as np

if "/opt/trn_rl_repo" not in sys.path:
    sys.path.insert(0, "/opt/trn_rl_repo")

B_PER_CORE = 2
N_CORES = 8
H = 512
W = 512
NBLK = H // 128
WPAD = W + 2  # zero column on each side

_compiled = None
_compiled_reps = {}


def _shift_mats():
    """[128, 5, 128] f32: j=0 I, 1 Sd (k=m-1), 2 Su (k=m+1), 3 Ed, 4 Eu."""
    m = np.zeros((128, 5, 128), dtype=np.float32)
    for i in range(128):
        m[i, 0, i] = 1.0  # identity
    for i in range(127):
        m[i, 1, i + 1] = 1.0  # Sd: out[m] = in[m-1]
        m[i + 1, 2, i] = 1.0  # Su: out[m] = in[m+1]
    m[127, 3, 0] = 1.0  # Ed: out[0] = in[127]   (prev block)
    m[0, 4, 127] = 1.0  # Eu: out[127] = in[0]   (next block)
    return m


def _build_program(reps=1):
    """reps>1 unrolls the whole per-core computation `reps` times inside one
    NEFF — used only to measure kernel time through the dispatch noise."""
    import concourse.bacc as bacc
    import concourse.mybir as mybir
    import concourse.tile as tile

    fp32 = mybir.dt.float32

    nc = bacc.Bacc(
        "TRN2",
        target_bir_lowering=False,
        debug=False,
        enable_asserts=False,
        num_devices=N_CORES,
    )

    aff_d = nc.dram_tensor(
        "affinity", [B_PER_CORE, 9, H, W], fp32, kind="ExternalInput"
    ).ap()
    cur_d = nc.dram_tensor(
        "cur_seg", [B_PER_CORE, 1, H, W], fp32, kind="ExternalInput"
    ).ap()
    coa_d = nc.dram_tensor(
        "coarse_seg", [B_PER_CORE, 1, H, W], fp32, kind="ExternalInput"
    ).ap()
    smat_d = nc.dram_tensor("smats", [128, 5, 128], fp32, kind="ExternalInput").ap()
    out_d = nc.dram_tensor(
        "out", [B_PER_CORE, 1, H, W], fp32, kind="ExternalOutput"
    ).ap()

    with tile.TileContext(nc) as tc:
        with (
            tc.tile_pool(name="smat", bufs=1) as smat_pool,
            tc.tile_pool(name="aff", bufs=8) as aff_pool,
            tc.tile_pool(name="prod", bufs=8) as prod_pool,
            tc.tile_pool(name="cur", bufs=2) as cur_pool,
            tc.tile_pool(name="coa", bufs=2) as coa_pool,
            tc.tile_pool(name="acc", bufs=2) as acc_pool,
            tc.tile_pool(name="psum", bufs=8, space="PSUM") as psum_pool,
        ):
            tS = smat_pool.tile([128, 5, 128], fp32)
            nc.scalar.dma_start(out=tS[:], in_=smat_d[:])
            SM_I, SM_SD, SM_SU, SM_ED, SM_EU = (tS[:, j, :] for j in range(5))

            for b in [bb for _ in range(reps) for bb in range(B_PER_CORE)]:
                # --- cur tile [128, 4, 514], data in cols 1..512 ---
                # cur/coarse/out ride the ACT HWDGE ring; affinity rides the
                # SP ring, so the two streams overlap.
                tM = cur_pool.tile([128, NBLK, WPAD], fp32, tag="cur")
                nc.vector.memset(tM[:, :, 0:1], 0.0)
                nc.vector.memset(tM[:, :, WPAD - 1 : WPAD], 0.0)
                nc.scalar.dma_start(
                    out=tM[:, :, 1 : W + 1],
                    in_=cur_d[b, 0].rearrange("(t p) c -> p t c", p=128),
                )

                tC = coa_pool.tile([128, NBLK, W], fp32, tag="coa")
                nc.scalar.dma_start(
                    out=tC[:], in_=coa_d[b, 0].rearrange("(t p) c -> p t c", p=128)
                )

                aff_flat = aff_d[b].flatten_outer_dims()  # [9*512, 512]

                # --- products and per-dy-group sums ---
                # adds interleave with muls (add1 overlaps the 3rd plane DMA);
                # last group's adds on DVE (idle at the tail), earlier groups
                # on GpSimd in parallel. Every 3rd plane loads via the ACT
                # ring to balance ring occupancy.
                V = []  # V[g] for dy = g-1
                for g in range(3):
                    dy = g - 1
                    add_eng = nc.vector if g == 2 else nc.gpsimd
                    Pg = []
                    for dxi in range(3):
                        k = 3 * g + dxi
                        dx = dxi - 1
                        # per-plane DMA, source rows shifted by -dy
                        ak = aff_pool.tile([128, NBLK, W], fp32, tag="aff")
                        start = 512 * k - dy
                        # group 2 rides SP entirely (idle by then);
                        # earlier groups send every 3rd plane via ACT
                        ring = nc.sync if g == 2 else (nc.scalar if dxi == 2 else nc.sync)
                        ring.dma_start(
                            out=ak[:],
                            in_=aff_flat[start : start + H, :].rearrange(
                                "(t p) c -> p t c", p=128
                            ),
                        )
                        pk = prod_pool.tile([128, NBLK, W], fp32, tag="prod")
                        src = tC[:] if k == 4 else tM[:, :, 1 + dx : 1 + dx + W]
                        nc.vector.tensor_mul(out=pk[:], in0=ak[:], in1=src)
                        Pg.append(pk)
                        if dxi == 1:
                            add_eng.tensor_add(
                                out=Pg[0][:], in0=Pg[0][:], in1=Pg[1][:]
                            )
                    add_eng.tensor_add(out=Pg[0][:], in0=Pg[0][:], in1=Pg[2][:])
                    V.append(Pg[0])

                    if g == 1:
                        # V_-1 and V_0 are ready: issue their matmuls now so
                        # only the V_+1-dependent ones remain in the tail.
                        Vm1, V0 = V[0], V[1]
                        psum_tiles = []
                        for t in range(NBLK):
                            pt = psum_pool.tile([128, W], fp32, tag="psum")
                            psum_tiles.append(pt)
                            nc.tensor.matmul(
                                pt[:], SM_I, V0[:, t, :], start=True, stop=False
                            )
                            nc.tensor.matmul(
                                pt[:], SM_SD, Vm1[:, t, :], start=False, stop=False
                            )
                            if t > 0:
                                nc.tensor.matmul(
                                    pt[:],
                                    SM_ED,
                                    Vm1[:, t - 1, :],
                                    start=False,
                                    stop=False,
                                )

                # --- tail: V_+1 matmuls, evacuate, store ---
                Vp1 = V[2]
                acc = acc_pool.tile([128, NBLK, W], fp32, tag="acc")
                out_blocks = out_d[b, 0].rearrange("(t p) c -> p t c", p=128)
                out_ring = nc.sync if b == B_PER_CORE - 1 else nc.scalar
                for t in range(NBLK):
                    pt = psum_tiles[t]
                    last_is_eu = t < NBLK - 1
                    nc.tensor.matmul(
                        pt[:], SM_SU, Vp1[:, t, :], start=False, stop=not last_is_eu
                    )
                    if last_is_eu:
                        nc.tensor.matmul(
                            pt[:], SM_EU, Vp1[:, t + 1, :], start=False, stop=True
                        )
                    nc.scalar.copy(out=acc[:, t, :], in_=pt[:])
                    out_ring.dma_start(out=out_blocks[:, t, :], in_=acc[:, t, :])

    nc.compile()
    return nc


def _get_program(reps=1):
    global _compiled
    if reps != 1:
        if reps not in _compiled_reps:
            _compiled_reps[reps] = _build_program(reps)
        return _compiled_reps[reps]
    if _compiled is None:
        _compiled = _build_program()
    return _compiled


def _in_maps(affinity, cur_seg, coarse_seg):
    smats = _shift_mats()
    maps = []
    for j in range(N_CORES):
        s = slice(j * B_PER_CORE, (j + 1) * B_PER_CORE)
        maps.append(
            {
                "affinity": np.ascontiguousarray(affinity[s]),
                "cur_seg": np.ascontiguousarray(cur_seg[s]),
                "coarse_seg": np.ascontiguousarray(coarse_seg[s]),
                "smats": smats,
            }
        )
    return maps


def kernel(affinity, cur_seg, coarse_seg, i=None, **_unused):
    from concourse.bass_utils import run_bass_kernel_spmd

    nc = _get_program()

    affinity = np.ascontiguousarray(affinity, dtype=np.float32)
    cur_seg = np.ascontiguousarray(cur_seg, dtype=np.float32)
    coarse_seg = np.ascontiguousarray(coarse_seg, dtype=np.float32)

    res = run_bass_kernel_spmd(
        nc, _in_maps(affinity, cur_seg, coarse_seg), core_ids=list(range(N_CORES))
    )
    out = np.concatenate([r["out"] for r in res.results], axis=0)
    return out
